# revision 12
# baseline (speedup 1.0000x reference)
"""Trainium2 Bass kernel for nn_D_GCN (Chebyshev-style GCN diffusion).

Reference computation (per batch b):
    x0 = X                       (T, N, F) node features
    x1 = A x0                    (diffusion over nodes)
    x2 = 2 A x1 - x0
    out = relu(stack_k(x_k) @ Theta1 + bias)     Theta row index = f*K + k

Algebraic refactoring (Theta_k := Theta1[k::3]):
    out = relu( g0 + A @ (h1 + A @ h2) )
    g0  = x0 (Theta_0 - Theta_2) + bias    [host, bf16]
    h1  = x0 Theta_1                       [host, fp8, x16]
    h2  = 2 x0 Theta_2                     [host, fp8]

Everything on device is computed TRANSPOSED (columns of the skinny
matrices on PSUM partitions) so the huge A matrix is always the
*moving* matmul operand at the fp8 free-dim maximum (rhs [128,2,512],
DoubleRow) while the skinny operand is stationary:
    pass 1:  w^T[c, m]  = h2^T A^T  (+ h1^T)      c = 128 output cols
    pass 2: out^T[c, n]  = w^T  A^T  (+ g0^T)
This doubles per-matmul MACs vs keeping A stationary (the baseline) and
moves the LDWEIGHTS cost to the small reused operand.

Sharding: 8 cores = 2 batches x (2 t-halves x 2 n-halves). A core's
c-columns are its 4 time steps x 32 output features; its n rows are its
2048-row output block. Pass 1 computes w^T for ALL 4096 m (2x
replicated within a batch across the n-halves - cheaper than any
collective on this runtime), pass 2 only the core's 2048 n columns.
w^T -> w (pass-2 stationary operand needs m on partitions) is done with
32 PE transposes via an fp8 identity, pipelined per 512-row block.

A^T is stored symmetrically slot-permuted (own n-half first on both
axes) so one resident 16 MiB fp8 tile serves pass 1 (all column blocks,
streamed&consumed in arrival order) and pass 2 (column blocks 0-3 =
own n columns; rows in the same slot order as w's chunks). The DMA
stream order IS the prefetch schedule: h2, A blocks (512 KiB pieces),
h1^T after the first block, g0^T last. Pass-2 matmuls for m-chunks of
block s issue as soon as block s is processed (s >= 3), so the PE
follows the A stream with a ~2.5 us tail after the last byte.
"""

import sys

if "/opt/trn_rl_repo" not in sys.path:
    sys.path.insert(0, "/opt/trn_rl_repo")

import numpy as np
import ml_dtypes

B, T, N, F, O = 2, 8, 4096, 32, 32
K = 3
NCORES = 8
NHALF = N // 2      # 2048 rows per output shard
C = 128             # output columns per core = 4 t-slices x 32 o
NCB = 8             # A^T column blocks of 512
CBW = 512           # column-block width
NPIECE = 4          # DMA pieces per column block (512 KiB each)
RCH = 32            # 128-row chunks of A^T
MCH = 32            # 128-row chunks of w

SCALE_A = 4096.0
SCALE_W = 16.0

_CACHE = {}


def _build_nc():
    import concourse.mybir as mybir
    import concourse.tile as tile
    from concourse import bacc, masks

    f32 = mybir.dt.float32
    bf16 = mybir.dt.bfloat16
    fp8 = mybir.dt.float8e4
    DR = mybir.MatmulPerfMode.DoubleRow

    nc = bacc.Bacc(None, num_devices=NCORES)

    # all inputs partition-major; A^T symmetrically slot-permuted
    A_d = nc.dram_tensor("A", [NCB, NPIECE, 128, RCH // NPIECE, CBW], fp8,
                         kind="ExternalInput")
    H2_d = nc.dram_tensor("H2", [128, RCH, C], fp8, kind="ExternalInput")
    H1_d = nc.dram_tensor("H1", [128, NCB, CBW], fp8, kind="ExternalInput")
    G0_d = nc.dram_tensor("G0", [128, NCB // 2, CBW], bf16,
                          kind="ExternalInput")
    OUT_d = nc.dram_tensor("OUT", [NCB // 2, 128, CBW], bf16,
                           kind="ExternalOutput")

    with tile.TileContext(nc) as tc:
        with (
            tc.tile_pool(name="big", bufs=1) as big,
            tc.tile_pool(name="stg", bufs=2) as stg,
            tc.tile_pool(name="ps", bufs=1, space="PSUM") as psp,
        ):
            A_sb = big.tile([128, NCB, RCH, CBW], fp8, name="Asb", tag="Asb")
            H2 = big.tile([128, RCH, C], fp8, name="H2s", tag="H2s")
            H1 = big.tile([128, NCB, CBW], fp8, name="H1s", tag="H1s")
            G0 = big.tile([128, NCB // 2, CBW], bf16, name="G0s", tag="G0s")
            W = big.tile([128, MCH, C], fp8, name="Ws", tag="Ws")
            OS = big.tile([128, NCB // 2, CBW], bf16, name="OSs", tag="OSs")
            ident = big.tile([128, 128], bf16, name="ident", tag="ident")

            masks.make_identity(nc, ident[:])

            # ---- one explicitly-ordered input stream on the SP ring ----
            nc.sync.dma_start(H2[:], H2_d[:])
            for pc in range(NPIECE):
                nc.sync.dma_start(
                    A_sb[:, 0, pc * 8:(pc + 1) * 8], A_d[0, pc])
            nc.sync.dma_start(H1[:], H1_d[:])
            for sb in range(1, NCB):
                for pc in range(NPIECE):
                    nc.sync.dma_start(
                        A_sb[:, sb, pc * 8:(pc + 1) * 8], A_d[sb, pc])
            nc.sync.dma_start(G0[:], G0_d[:])

            # ---- PE warm-up (HAM clock gate): dummy matmuls while the
            # first DMAs land so the real matmuls start at 2.4 GHz. Lands
            # in the psum bank pass 1 re-opens with start=True.
            warm_src = big.tile([128, 2, CBW], fp8, name="warmsrc",
                                tag="warmsrc")
            nc.gpsimd.memset(warm_src[:], 0.0)
            warm_ps = psp.tile([128, CBW], f32, name="warm", tag="bank4")
            NWARM = 16
            for wi in range(NWARM):
                nc.tensor.matmul(
                    warm_ps[:, 0:256], warm_src[:, :, 0:128],
                    warm_src[:, :, 0:256],
                    start=(wi == 0), stop=(wi == NWARM - 1), perf_mode=DR)

            # pass-2 psum banks accumulate across the whole stream
            ps2 = [psp.tile([128, CBW], f32, name=f"o{nb}", tag=f"bank{nb}")
                   for nb in range(4)]
            p2_count = [0] * 4

            def p2_mm(mp, nb):
                nc.tensor.matmul(
                    ps2[nb][:],
                    W[:, 2 * mp:2 * mp + 2],
                    A_sb[:, nb, 2 * mp:2 * mp + 2],
                    start=(p2_count[nb] == 0),
                    stop=(p2_count[nb] == MCH // 2 - 1),
                    perf_mode=DR)
                p2_count[nb] += 1

            Relu = mybir.ActivationFunctionType.Relu
            with nc.named_scope("main"):
                for s in range(NCB):
                    # pass 1: w^T columns for block s (512 slot-rows of w)
                    ps1 = psp.tile([128, CBW], f32, name=f"y{s % 2}",
                                   tag=f"bank{4 + s % 2}")
                    for lp in range(RCH // 2):
                        nc.tensor.matmul(
                            ps1[:], H2[:, 2 * lp:2 * lp + 2],
                            A_sb[:, s, 2 * lp:2 * lp + 2],
                            start=(lp == 0), stop=(lp == RCH // 2 - 1),
                            perf_mode=DR)
                    # w^T = psum*(16/4096) + h1^T*16   [bf16 staging]
                    wt = stg.tile([128, CBW], bf16, name="wt", tag="wt")
                    nc.vector.scalar_tensor_tensor(
                        wt[:], ps1[:], 1.0 / 256.0, H1[:, s],
                        mybir.AluOpType.mult, mybir.AluOpType.add)
                    # transpose to w[m, c], 4 x 128x128 blocks via PE
                    # (bf16: fp8 PE-transpose needs out elem step 2);
                    # the psum->W copy casts to fp8 for the pass-2 lhsT
                    for tb in range(4):
                        pst = psp.tile([128, 128], bf16, name=f"t{tb % 2}",
                                       tag=f"bank{6 + tb % 2}")
                        nc.tensor.transpose(
                            pst[:], wt[:, 128 * tb:128 * (tb + 1)], ident[:])
                        nc.vector.tensor_copy(W[:, 4 * s + tb], pst[:])
                    # pass 2: emit each (w m-pair, n-block) matmul as soon
                    # as both the rhs block (nb <= s, first 4 = own n cols)
                    # and the transposed w chunks (slots <= s) exist, so
                    # the PE never piles up behind the stream
                    for mp in range(2 * s, 2 * s + 2):
                        for nb in range(min(s + 1, 4)):
                            p2_mm(mp, nb)
                    if s < 4:
                        for mp in range(2 * s):
                            p2_mm(mp, s)
                for nb in range(4):
                    # out^T = psum/(SCALE_A*SCALE_W) + g0^T, relu
                    nc.vector.scalar_tensor_tensor(
                        OS[:, nb], ps2[nb][:], 1.0 / 65536.0, G0[:, nb],
                        mybir.AluOpType.mult, mybir.AluOpType.add)
                    nc.scalar.activation(OS[:, nb], OS[:, nb], Relu)
                    nc.scalar.dma_start(OUT_d[nb], OS[:, nb])

    nc.compile()
    return nc


def _get_nc():
    if "nc" not in _CACHE:
        _CACHE["nc"] = _build_nc()
    return _CACHE["nc"]


def _prepare_in_maps(X, A_q, Theta1, bias):
    fp8 = ml_dtypes.float8_e4m3
    bf16 = ml_dtypes.bfloat16
    X = np.asarray(X, dtype=np.float32)
    A_q = np.asarray(A_q, dtype=np.float32)
    Theta1 = np.asarray(Theta1, dtype=np.float32)
    bias = np.asarray(bias, dtype=np.float32)

    Th = Theta1.reshape(F, K, O)
    Th0, Th1, Th2 = Th[:, 0], Th[:, 1], Th[:, 2]

    # 4 unique permuted A^T tiles (batch x n-half), shared by t-halves
    A_tiles = {}
    for b in range(B):
        At = (A_q[b].T * SCALE_A).astype(fp8)        # [l/m, m/n]
        for h in range(2):
            if h == 1:
                Ats = np.empty_like(At)
                Ats[:NHALF, :NHALF] = At[NHALF:, NHALF:]
                Ats[:NHALF, NHALF:] = At[NHALF:, :NHALF]
                Ats[NHALF:, :NHALF] = At[:NHALF, NHALF:]
                Ats[NHALF:, NHALF:] = At[:NHALF, :NHALF]
            else:
                Ats = At
            # [row, col] -> [cb, piece, p, rc', q]
            A_tiles[b, h] = np.ascontiguousarray(
                Ats.reshape(RCH, 128, NCB, CBW)
                .transpose(2, 0, 1, 3)                 # [cb, rc, p, q]
                .reshape(NCB, NPIECE, RCH // NPIECE, 128, CBW)
                .transpose(0, 1, 3, 2, 4))             # [cb, pc, p, rc', q]

    in_maps = []
    for core in range(NCORES):
        b, th, h = core // 4, (core // 2) % 2, core % 2
        Xb = X[b, 4 * th:4 * th + 4]                   # (4, N, F)
        sig = np.r_[np.arange(NHALF * h, NHALF * (h + 1)),
                    np.arange(0, NHALF * h), np.arange(NHALF * (h + 1), N)]
        # skinny mats, c = 32*t_rel + o on the trailing axis -> (N, 128)
        h2 = np.transpose(2.0 * (Xb @ Th2), (1, 0, 2)).reshape(N, C)[sig]
        h1 = np.transpose(Xb @ Th1, (1, 0, 2)).reshape(N, C)[sig]
        g0 = (np.transpose(Xb @ (Th0 - Th2), (1, 0, 2)).reshape(N, C)
              + np.tile(bias, 4)[np.newaxis, :])[NHALF * h:NHALF * (h + 1)]
        in_maps.append({
            "A": A_tiles[b, h],
            "H2": np.ascontiguousarray(
                h2.reshape(RCH, 128, C).transpose(1, 0, 2)).astype(fp8),
            "H1": np.ascontiguousarray(
                (SCALE_W * h1).reshape(NCB, CBW, C)
                .transpose(2, 0, 1)).astype(fp8),
            "G0": np.ascontiguousarray(
                g0.reshape(NCB // 2, CBW, C).transpose(2, 0, 1)).astype(bf16),
        })
    return in_maps


def run_with_results(inputs, **spmd_kwargs):
    """Returns (full_output, BassKernelResults). spmd_kwargs forwarded to
    run_bass_kernel_spmd (e.g. trace=True)."""
    from concourse.bass_utils import run_bass_kernel_spmd

    nc = _get_nc()
    in_maps = _prepare_in_maps(**inputs)
    res = run_bass_kernel_spmd(
        nc, in_maps, core_ids=list(range(NCORES)), **spmd_kwargs)

    out = np.empty((B, T, N, O), dtype=np.float32)
    for core in range(NCORES):
        b, th, h = core // 4, (core // 2) % 2, core % 2
        blk = np.asarray(res.results[core]["OUT"], dtype=np.float32)
        # [nb, p, q] -> [p, nb*q] -> [t_rel, o, n_local] -> [t, n, o]
        arr = blk.transpose(1, 0, 2).reshape(4, O, NHALF)
        out[b, 4 * th:4 * th + 4, NHALF * h:NHALF * (h + 1), :] = (
            arr.transpose(0, 2, 1))
    return out, res


def kernel(X, A_q, Theta1, bias):
    out, _ = run_with_results(
        {"X": X, "A_q": A_q, "Theta1": Theta1, "bias": bias})
    return out


# revision 13
# speedup vs baseline: 1.0131x; 1.0131x over previous
"""Trainium2 Bass kernel for nn_D_GCN (Chebyshev-style GCN diffusion).

Reference computation (per batch b):
    x0 = X                       (T, N, F) node features
    x1 = A x0                    (diffusion over nodes)
    x2 = 2 A x1 - x0
    out = relu(stack_k(x_k) @ Theta1 + bias)     Theta row index = f*K + k

Algebraic refactoring (Theta_k := Theta1[k::3]):
    out = relu( g0 + A @ (h1 + A @ h2) )
    g0  = x0 (Theta_0 - Theta_2) + bias    [host, bf16]
    h1  = x0 Theta_1                       [host, fp8, x16]
    h2  = 2 x0 Theta_2                     [host, fp8]

Everything on device is computed TRANSPOSED (columns of the skinny
matrices on PSUM partitions) so the huge A matrix is always the
*moving* matmul operand at the fp8 free-dim maximum (rhs [128,2,512],
DoubleRow) while the skinny operand is stationary:
    pass 1:  w^T[c, m]  = h2^T A^T  (+ h1^T)      c = 128 output cols
    pass 2: out^T[c, n]  = w^T  A^T  (+ g0^T)
This doubles per-matmul MACs vs keeping A stationary (the baseline) and
moves the LDWEIGHTS cost to the small reused operand.

Sharding: 8 cores = 2 batches x (2 t-halves x 2 n-halves). A core's
c-columns are its 4 time steps x 32 output features; its n rows are its
2048-row output block. Pass 1 computes w^T for ALL 4096 m (2x
replicated within a batch across the n-halves - cheaper than any
collective on this runtime), pass 2 only the core's 2048 n columns.
w^T -> w (pass-2 stationary operand needs m on partitions) is done with
32 PE transposes via an fp8 identity, pipelined per 512-row block.

A^T is stored symmetrically slot-permuted (own n-half first on both
axes) so one resident 16 MiB fp8 tile serves pass 1 (all column blocks,
streamed&consumed in arrival order) and pass 2 (column blocks 0-3 =
own n columns; rows in the same slot order as w's chunks). The DMA
stream order IS the prefetch schedule: h2, A blocks (512 KiB pieces),
h1^T after the first block, g0^T last. Pass-2 matmuls for m-chunks of
block s issue as soon as block s is processed (s >= 3), so the PE
follows the A stream with a ~2.5 us tail after the last byte.
"""

import sys

if "/opt/trn_rl_repo" not in sys.path:
    sys.path.insert(0, "/opt/trn_rl_repo")

import numpy as np
import ml_dtypes

B, T, N, F, O = 2, 8, 4096, 32, 32
K = 3
NCORES = 8
NHALF = N // 2      # 2048 rows per output shard
C = 128             # output columns per core = 4 t-slices x 32 o
NCB = 8             # A^T column blocks of 512
CBW = 512           # column-block width
NPIECE = 4          # DMA pieces per column block (512 KiB each)
RCH = 32            # 128-row chunks of A^T
MCH = 32            # 128-row chunks of w

SCALE_A = 4096.0
SCALE_W = 16.0

_CACHE = {}


def _build_nc():
    import concourse.mybir as mybir
    import concourse.tile as tile
    from concourse import bacc, masks

    f32 = mybir.dt.float32
    bf16 = mybir.dt.bfloat16
    fp8 = mybir.dt.float8e4
    DR = mybir.MatmulPerfMode.DoubleRow

    nc = bacc.Bacc(None, num_devices=NCORES)

    # all inputs partition-major; A^T symmetrically slot-permuted
    A_d = nc.dram_tensor("A", [NCB, NPIECE, 128, RCH // NPIECE, CBW], fp8,
                         kind="ExternalInput")
    H2_d = nc.dram_tensor("H2", [128, RCH, C], fp8, kind="ExternalInput")
    H1_d = nc.dram_tensor("H1", [128, NCB, CBW], fp8, kind="ExternalInput")
    G0_d = nc.dram_tensor("G0", [128, NCB // 2, CBW], bf16,
                          kind="ExternalInput")
    OUT_d = nc.dram_tensor("OUT", [NCB // 2, 128, CBW], bf16,
                           kind="ExternalOutput")

    with tile.TileContext(nc) as tc:
        with (
            tc.tile_pool(name="big", bufs=1) as big,
            tc.tile_pool(name="stg", bufs=2) as stg,
            tc.tile_pool(name="ps", bufs=1, space="PSUM") as psp,
        ):
            A_sb = big.tile([128, NCB, RCH, CBW], fp8, name="Asb", tag="Asb")
            H2 = big.tile([128, RCH, C], fp8, name="H2s", tag="H2s")
            H1 = big.tile([128, NCB, CBW], fp8, name="H1s", tag="H1s")
            G0 = big.tile([128, NCB // 2, CBW], bf16, name="G0s", tag="G0s")
            W = big.tile([128, MCH, C], fp8, name="Ws", tag="Ws")
            OS = big.tile([128, NCB // 2, CBW], bf16, name="OSs", tag="OSs")
            ident = big.tile([128, 128], bf16, name="ident", tag="ident")

            masks.make_identity(nc, ident[:])

            # ---- one explicitly-ordered input stream on the SP ring ----
            nc.sync.dma_start(H2[:], H2_d[:])
            for pc in range(NPIECE):
                nc.sync.dma_start(
                    A_sb[:, 0, pc * 8:(pc + 1) * 8], A_d[0, pc])
            nc.sync.dma_start(H1[:], H1_d[:])
            for sb in range(1, NCB):
                for pc in range(NPIECE):
                    nc.sync.dma_start(
                        A_sb[:, sb, pc * 8:(pc + 1) * 8], A_d[sb, pc])
            nc.sync.dma_start(G0[:], G0_d[:])

            # ---- PE warm-up (HAM clock gate): dummy matmuls while the
            # first DMAs land so the real matmuls start at 2.4 GHz. Lands
            # in the psum bank pass 1 re-opens with start=True.
            warm_src = big.tile([128, 2, CBW], fp8, name="warmsrc",
                                tag="warmsrc")
            nc.gpsimd.memset(warm_src[:], 0.0)
            warm_ps = psp.tile([128, CBW], f32, name="warm", tag="bank4")
            NWARM = 16
            for wi in range(NWARM):
                nc.tensor.matmul(
                    warm_ps[:, 0:256], warm_src[:, :, 0:128],
                    warm_src[:, :, 0:256],
                    start=(wi == 0), stop=(wi == NWARM - 1), perf_mode=DR)

            # pass-2 psum banks accumulate across the whole stream
            ps2 = [psp.tile([128, CBW], f32, name=f"o{nb}", tag=f"bank{nb}")
                   for nb in range(4)]
            p2_count = [0] * 4

            def p2_mm(mp, nb):
                nc.tensor.matmul(
                    ps2[nb][:],
                    W[:, 2 * mp:2 * mp + 2],
                    A_sb[:, nb, 2 * mp:2 * mp + 2],
                    start=(p2_count[nb] == 0),
                    stop=(p2_count[nb] == MCH // 2 - 1),
                    perf_mode=DR)
                p2_count[nb] += 1

            def transp(wt, s, tb):
                # w^T -> w[m, c] 128x128 block via PE (bf16: fp8
                # PE-transpose needs out elem step 2); the psum->W copy
                # casts to fp8 for the pass-2 lhsT
                pst = psp.tile([128, 128], bf16, name=f"t{tb % 2}",
                               tag=f"bank{6 + tb % 2}")
                nc.tensor.transpose(
                    pst[:], wt[:, 128 * tb:128 * (tb + 1)], ident[:])
                nc.vector.tensor_copy(W[:, 4 * s + tb], pst[:])

            Relu = mybir.ActivationFunctionType.Relu
            with nc.named_scope("main"):
                # pend: PE work of slot s-1 (transposes + pass-2 matmuls),
                # emitted BETWEEN slot-s p1 piece groups so the strict-FIFO
                # PE queue always has runnable fillers ahead of a
                # DMA-stalled matmul; the last slot then leaves almost no
                # post-stream backlog.
                pend = []
                for s in range(NCB):
                    ps1 = psp.tile([128, CBW], f32, name=f"y{s % 2}",
                                   tag=f"bank{4 + s % 2}")
                    for pc in range(NPIECE):
                        for lp in range(4 * pc, 4 * pc + 4):
                            nc.tensor.matmul(
                                ps1[:], H2[:, 2 * lp:2 * lp + 2],
                                A_sb[:, s, 2 * lp:2 * lp + 2],
                                start=(lp == 0), stop=(lp == RCH // 2 - 1),
                                perf_mode=DR)
                        take = -(-len(pend) // (NPIECE - pc))
                        for op in pend[:take]:
                            op()
                        pend = pend[take:]
                    # w^T = psum*(16/4096) + h1^T*16   [bf16 staging]
                    wt = stg.tile([128, CBW], bf16, name="wt", tag="wt")
                    nc.vector.scalar_tensor_tensor(
                        wt[:], ps1[:], 1.0 / 256.0, H1[:, s],
                        mybir.AluOpType.mult, mybir.AluOpType.add)
                    if s == NCB - 1:
                        break
                    # build slot-s leftovers: 4 transposes + every pass-2
                    # matmul whose inputs (rhs block nb, w chunks) now exist
                    pend = [lambda wt=wt, s=s, tb=tb: transp(wt, s, tb)
                            for tb in range(4)]
                    for mp in range(2 * s, 2 * s + 2):
                        for nb in range(min(s + 1, 4)):
                            pend.append(lambda mp=mp, nb=nb: p2_mm(mp, nb))
                    if s < 4:
                        for mp in range(2 * s):
                            pend.append(lambda mp=mp, s=s: p2_mm(mp, s))
                # tail: last slot's transposes, then nb-major pass-2 pairs
                # with each bank's drain chain right behind its stop
                s = NCB - 1
                for tb in range(4):
                    transp(wt, s, tb)
                for nb in range(4):
                    p2_mm(2 * s, nb)
                    p2_mm(2 * s + 1, nb)
                    # out^T = psum/(SCALE_A*SCALE_W) + g0^T, relu
                    nc.vector.scalar_tensor_tensor(
                        OS[:, nb], ps2[nb][:], 1.0 / 65536.0, G0[:, nb],
                        mybir.AluOpType.mult, mybir.AluOpType.add)
                    nc.scalar.activation(OS[:, nb], OS[:, nb], Relu)
                    nc.scalar.dma_start(OUT_d[nb], OS[:, nb])

    nc.compile()
    return nc


def _get_nc():
    if "nc" not in _CACHE:
        _CACHE["nc"] = _build_nc()
    return _CACHE["nc"]


def _prepare_in_maps(X, A_q, Theta1, bias):
    fp8 = ml_dtypes.float8_e4m3
    bf16 = ml_dtypes.bfloat16
    X = np.asarray(X, dtype=np.float32)
    A_q = np.asarray(A_q, dtype=np.float32)
    Theta1 = np.asarray(Theta1, dtype=np.float32)
    bias = np.asarray(bias, dtype=np.float32)

    Th = Theta1.reshape(F, K, O)
    Th0, Th1, Th2 = Th[:, 0], Th[:, 1], Th[:, 2]

    # 4 unique permuted A^T tiles (batch x n-half), shared by t-halves
    A_tiles = {}
    for b in range(B):
        At = (A_q[b].T * SCALE_A).astype(fp8)        # [l/m, m/n]
        for h in range(2):
            if h == 1:
                Ats = np.empty_like(At)
                Ats[:NHALF, :NHALF] = At[NHALF:, NHALF:]
                Ats[:NHALF, NHALF:] = At[NHALF:, :NHALF]
                Ats[NHALF:, :NHALF] = At[:NHALF, NHALF:]
                Ats[NHALF:, NHALF:] = At[:NHALF, :NHALF]
            else:
                Ats = At
            # [row, col] -> [cb, piece, p, rc', q]
            A_tiles[b, h] = np.ascontiguousarray(
                Ats.reshape(RCH, 128, NCB, CBW)
                .transpose(2, 0, 1, 3)                 # [cb, rc, p, q]
                .reshape(NCB, NPIECE, RCH // NPIECE, 128, CBW)
                .transpose(0, 1, 3, 2, 4))             # [cb, pc, p, rc', q]

    in_maps = []
    for core in range(NCORES):
        b, th, h = core // 4, (core // 2) % 2, core % 2
        Xb = X[b, 4 * th:4 * th + 4]                   # (4, N, F)
        sig = np.r_[np.arange(NHALF * h, NHALF * (h + 1)),
                    np.arange(0, NHALF * h), np.arange(NHALF * (h + 1), N)]
        # skinny mats, c = 32*t_rel + o on the trailing axis -> (N, 128)
        h2 = np.transpose(2.0 * (Xb @ Th2), (1, 0, 2)).reshape(N, C)[sig]
        h1 = np.transpose(Xb @ Th1, (1, 0, 2)).reshape(N, C)[sig]
        g0 = (np.transpose(Xb @ (Th0 - Th2), (1, 0, 2)).reshape(N, C)
              + np.tile(bias, 4)[np.newaxis, :])[NHALF * h:NHALF * (h + 1)]
        in_maps.append({
            "A": A_tiles[b, h],
            "H2": np.ascontiguousarray(
                h2.reshape(RCH, 128, C).transpose(1, 0, 2)).astype(fp8),
            "H1": np.ascontiguousarray(
                (SCALE_W * h1).reshape(NCB, CBW, C)
                .transpose(2, 0, 1)).astype(fp8),
            "G0": np.ascontiguousarray(
                g0.reshape(NCB // 2, CBW, C).transpose(2, 0, 1)).astype(bf16),
        })
    return in_maps


def run_with_results(inputs, **spmd_kwargs):
    """Returns (full_output, BassKernelResults). spmd_kwargs forwarded to
    run_bass_kernel_spmd (e.g. trace=True)."""
    from concourse.bass_utils import run_bass_kernel_spmd

    nc = _get_nc()
    in_maps = _prepare_in_maps(**inputs)
    res = run_bass_kernel_spmd(
        nc, in_maps, core_ids=list(range(NCORES)), **spmd_kwargs)

    out = np.empty((B, T, N, O), dtype=np.float32)
    for core in range(NCORES):
        b, th, h = core // 4, (core // 2) % 2, core % 2
        blk = np.asarray(res.results[core]["OUT"], dtype=np.float32)
        # [nb, p, q] -> [p, nb*q] -> [t_rel, o, n_local] -> [t, n, o]
        arr = blk.transpose(1, 0, 2).reshape(4, O, NHALF)
        out[b, 4 * th:4 * th + 4, NHALF * h:NHALF * (h + 1), :] = (
            arr.transpose(0, 2, 1))
    return out, res


def kernel(X, A_q, Theta1, bias):
    out, _ = run_with_results(
        {"X": X, "A_q": A_q, "Theta1": Theta1, "bias": bias})
    return out
